# revision 8
# baseline (speedup 1.0000x reference)
"""HGT layer on 8 Trainium2 NeuronCores (Bass/Tile, SPMD).

Strategy (dst-partitioned, per the sharding hint's second option):
  - Destination nodes of each ntype are partitioned contiguously across the
    8 cores (6250 nodes/core, padded to 6272 = 49*128).
  - Phase A (dense): each core computes K/V projections for ITS node slice
    (per-head watt/wmsg folded into the weights on the host, so K/V are
    plain [128,128] matmuls), plus the Q projection for its dst slice
    (mu/sqrt(dk) folded into Wq).  K and V are packed into one fused
    KV[6272,256] f32 table row so the edge phase gathers both in one DMA.
  - AllGather (one per etype) replicates the KV tables to all cores.
  - Phase B (edges): edges are routed by dst to the owning core and sorted
    by dst on the host; each 128-node dst-tile owns up to 14*128 = 1792
    edge slots (padded; verified max 1662 on these shapes).  Per 128-edge
    subtile: one indirect-DMA gather of KV rows (1KB), one of Q rows, then
    score = rowreduce(k*q), w = exp(score) (scores are O(0.6) so softmax
    needs no max-subtraction), a dst one-hot via iota/is_equal, and one
    matmul accumulating both the weighted messages AND the softmax
    denominator ([128 nodes x 132] PSUM: 128 msg cols + 4 den cols).
  - Phase C (node out): h = msg/den, out = h @ (alpha*Wa) + (1-alpha)*feat
    + alpha*ba (host-folded), then LayerNorm.

Transfer budget (the axon tunnel is ~37 MB/s with ~70 ms/array overhead),
so the host<->device traffic is consolidated into 4 arrays per call:
  - "io" [2,6272,128] fp16 per core: uploaded with the node features and
    DONATED as the output buffer (the kernel reads feats from it during
    phases A/C and overwrites it with the result in phase C; every read
    transitively precedes every write through the KV/Q/H dependency
    chain, and Tile enforces per-region WAR ordering on the same tensor).
  - "wd" fp16 flat: all folded weights/biases/LN params.
  - "sidx" int32 / "oidx" uint8: edge source-row and one-hot ids (the Q
    gather rows are derived on device as (oidx & 127) + 128*dst_tile).

Self-contained: hardcoded for N=50000, E=600000, D=128, H=4, DK=32, 8 cores.
"""
import math
import numpy as np

N = 50000
C = 8
NL = N // C            # 6250 nodes per core
DT = 49                # dst tiles of 128 per core
NLP = DT * 128         # 6272 padded local nodes
TPD = 14               # edge subtiles (of 128) per dst tile
T = DT * TPD           # 686 edge subtiles per etype per core
H_, DK, D = 4, 32, 128
LN_EPS = 1e-5
WMAT = 16384           # one [128,128] in the wd blob
R0 = 8 * WMAT          # row section base
WD_LEN = R0 + 13 * D

_BUILT = None          # compiled runner cached per process
_CACHE_FILE = "/root/.cache/hgt_trn2_kernel_v1.pkl"


# ----------------------------------------------------------------- host prep
def _blockdiag(w):
    out = np.zeros((D, D), np.float32)
    for h in range(H_):
        out[h * DK:(h + 1) * DK, h * DK:(h + 1) * DK] = w[h]
    return out


def _edge_prep(src, dst):
    """Route edges by dst core, sort by dst, pad per dst-tile to TPD*128.

    Returns per-core [128, T] arrays: src row id into the padded global KV
    table (int32, pad=0) and one-hot id (uint8, 0..127, pad=255).
    """
    core = dst // NL
    loc = dst - core * NL
    bucket = core * DT + (loc >> 7)
    order = np.argsort(dst, kind="stable")
    b_sorted = bucket[order]
    cnt = np.bincount(bucket, minlength=C * DT)
    if cnt.max() > TPD * 128:
        raise OverflowError("dst-tile overflow")
    starts = np.zeros(C * DT, np.int64)
    np.cumsum(cnt[:-1], out=starts[1:])
    rank = np.arange(len(dst)) - starts[b_sorted]
    # scatter straight into the device layout [C, 128, T]:
    # element (core, edge_in_subtile, dst_tile*TPD + subtile)
    t_g = (b_sorted % DT) * TPD + (rank >> 7)
    pos = (b_sorted // DT) * (128 * T) + (rank & 127) * T + t_g

    src_s = src[order]
    src_pad_id = ((src_s // NL) * NLP + (src_s % NL)).astype(np.int32)
    loc_s = loc[order]

    sp = np.zeros(C * 128 * T, np.int32)
    op = np.full(C * 128 * T, 255, np.uint8)
    sp[pos] = src_pad_id
    op[pos] = (loc_s & 127).astype(np.uint8)
    return sp.reshape(C, 128, T), op.reshape(C, 128, T)


# --------------------------------------------------------------- bass build
def _build():
    import jax
    from jax.experimental.shard_map import shard_map
    from jax.sharding import Mesh, PartitionSpec
    import concourse.bass as bass
    import concourse.bacc as bacc
    import concourse.mybir as mybir
    import concourse.tile as tile
    from concourse import bass2jax as b2j
    from concourse.masks import make_identity

    f32, f16, bf16 = mybir.dt.float32, mybir.dt.float16, mybir.dt.bfloat16
    u8, u16, i32 = mybir.dt.uint8, mybir.dt.uint16, mybir.dt.int32
    AF = mybir.ActivationFunctionType
    OP = mybir.AluOpType

    nc = bacc.Bacc("TRN2", target_bir_lowering=False, debug=False,
                   num_devices=C)
    wd = nc.declare_dram_parameter("wd", [WD_LEN], f16, isOutput=False)
    sidx = nc.declare_dram_parameter("sidx", [128, 2 * T], i32, isOutput=False)
    oidx = nc.declare_dram_parameter("oidx", [128, 2 * T], u8, isOutput=False)
    io = nc.declare_dram_parameter("io", [2, NLP, D], f16, isOutput=True)

    kv_loc = {e: nc.dram_tensor(f"kvloc_{e}", [NLP, 2 * D], f32)
              for e in ("ui", "iu")}
    kv_full = {e: nc.dram_tensor(f"kvfull_{e}", [C * NLP, 2 * D], f32,
                                 addr_space="Shared") for e in ("ui", "iu")}
    qtab = {e: nc.dram_tensor(f"qtab_{e}", [NLP, D], f32) for e in ("ui", "iu")}
    htab = {e: nc.dram_tensor(f"htab_{e}", [NLP, D], f16) for e in ("ui", "iu")}

    def wmat(m):   # [128,128] f16 view into wd
        return wd[m * WMAT:(m + 1) * WMAT].rearrange("(a b) -> a b", b=D)

    def wrow(r):   # [1,128] f16 view
        return wd[R0 + r * D:R0 + (r + 1) * D].rearrange("(a b) -> a b", a=1)

    WNAMES = ("Wk_ui", "Wv_ui", "Wq_ui", "Wk_iu", "Wv_iu", "Wq_iu",
              "Wa_u", "Wa_i")
    RNAMES = ("bk_ui", "bv_ui", "bq_ui", "bk_iu", "bv_iu", "bq_iu",
              "g_u", "b_u", "g_i", "b_i", "bam_u", "bam_i")

    with tile.TileContext(nc) as tc:
        with (tc.tile_pool(name="pers", bufs=1) as pers,
              tc.tile_pool(name="wk", bufs=3) as wk,
              tc.tile_pool(name="eg", bufs=3) as eg,
              tc.tile_pool(name="psA", bufs=2, space="PSUM") as psA,
              tc.tile_pool(name="psB", bufs=2, space="PSUM") as psB):
            ident = pers.tile([128, 128], f16)
            make_identity(nc, ident[:])
            iota = pers.tile([128, 128], u16)
            nc.gpsimd.iota(iota[:], pattern=[[1, 128]], base=0,
                           channel_multiplier=0)
            epst = pers.tile([128, 1], f32)
            nc.vector.memset(epst[:], LN_EPS)
            Wsb = {}
            for m, n in enumerate(WNAMES):
                Wsb[n] = pers.tile([128, D], f16, tag=f"W_{n}", name=f"W_{n}")
                nc.sync.dma_start(out=Wsb[n][:], in_=wmat(m))
            Bsb = {}
            for r, n in enumerate(RNAMES):
                b16 = pers.tile([128, D], f16, tag=f"B16_{n}", name=f"B16_{n}")
                nc.sync.dma_start(out=b16[:], in_=wrow(r).to_broadcast([128, D]))
                Bsb[n] = pers.tile([128, D], f32, tag=f"B_{n}", name=f"B_{n}")
                nc.vector.tensor_copy(out=Bsb[n][:], in_=b16[:])
            alt = {}
            for i, n in enumerate(("au", "ai")):
                a16 = pers.tile([128, 1], f16, tag=f"a16_{n}", name=f"a16_{n}")
                nc.sync.dma_start(
                    out=a16[:],
                    in_=wd[R0 + 12 * D + i:R0 + 12 * D + i + 1]
                    .rearrange("(a b) -> a b", a=1).to_broadcast([128, 1]))
                alt[n] = pers.tile([128, 1], f32, tag=f"al_{n}", name=f"al_{n}")
                nc.vector.tensor_copy(out=alt[n][:], in_=a16[:])
            six = pers.tile([128, 2 * T], i32)
            nc.sync.dma_start(out=six[:], in_=sidx[:])
            o8t = pers.tile([128, 2 * T], u8)
            nc.sync.dma_start(out=o8t[:], in_=oidx[:])
            o16 = pers.tile([128, 2 * T], u16)
            nc.vector.tensor_copy(out=o16[:], in_=o8t[:])
            o16m = pers.tile([128, 2 * T], u16)
            nc.vector.tensor_scalar(out=o16m[:], in0=o16[:], scalar1=127,
                                    scalar2=None, op0=OP.bitwise_and)

            # ---------------- Phase A: projection tables ----------------
            def phase_a(nt, e_src, e_dst):
                for t in range(DT):
                    fin = wk.tile([128, 128], f16, tag="fin")
                    nc.sync.dma_start(out=fin[:],
                                      in_=io[nt, t * 128:(t + 1) * 128, :])
                    psT = psA.tile([128, 128], f16, space="PSUM", tag="psT")
                    nc.tensor.transpose(out=psT[:], in_=fin[:],
                                        identity=ident[:])
                    fT = wk.tile([128, 128], f16, tag="fT")
                    nc.vector.tensor_copy(out=fT[:], in_=psT[:])
                    for role, wname, bname, dst_ap in (
                        ("k", f"Wk_{e_src}", f"bk_{e_src}",
                         kv_loc[e_src][t * 128:(t + 1) * 128, 0:D]),
                        ("v", f"Wv_{e_src}", f"bv_{e_src}",
                         kv_loc[e_src][t * 128:(t + 1) * 128, D:2 * D]),
                        ("q", f"Wq_{e_dst}", f"bq_{e_dst}",
                         qtab[e_dst][t * 128:(t + 1) * 128, :]),
                    ):
                        ps = psA.tile([128, 128], f32, space="PSUM",
                                      tag="psa", bufs=3, name=f"ps_{role}")
                        nc.tensor.matmul(out=ps[:], lhsT=fT[:],
                                         rhs=Wsb[wname][:], start=True,
                                         stop=True)
                        sb = wk.tile([128, 128], f32, tag=f"sb_{role}")
                        nc.vector.tensor_tensor(out=sb[:], in0=ps[:],
                                                in1=Bsb[bname][:], op=OP.add)
                        nc.sync.dma_start(out=dst_ap, in_=sb[:])

            phase_a(0, "ui", "iu")   # user feats: K/V for ui, Q for iu
            nc.gpsimd.collective_compute(
                "AllGather", OP.bypass, replica_groups=[list(range(C))],
                ins=[kv_loc["ui"][:]], outs=[kv_full["ui"][:]])
            phase_a(1, "iu", "ui")
            nc.gpsimd.collective_compute(
                "AllGather", OP.bypass, replica_groups=[list(range(C))],
                ins=[kv_loc["iu"][:]], outs=[kv_full["iu"][:]])

            # ---------------- Phase B: edge aggregation ----------------
            def phase_b(e, ecol):
                c0 = ecol * T
                for d in range(DT):
                    q32 = eg.tile([128, TPD], i32, tag="q32")
                    nc.vector.tensor_scalar(
                        out=q32[:], in0=o16m[:, c0 + d * TPD:c0 + (d + 1) * TPD],
                        scalar1=d * 128, scalar2=None, op0=OP.add)
                    pmsg = psB.tile([128, 132], f32, space="PSUM", tag="pmsg")
                    for b2 in range(2):
                        j0 = c0 + d * TPD + b2 * 7
                        kvg = eg.tile([128, 7, 2 * D], f32, tag="kvg")
                        qg = eg.tile([128, 7, D], f32, tag="qg")
                        for jj in range(7):
                            nc.gpsimd.indirect_dma_start(
                                out=kvg[:, jj, :], out_offset=None,
                                in_=kv_full[e][:],
                                in_offset=bass.IndirectOffsetOnAxis(
                                    ap=six[:, j0 + jj:j0 + jj + 1], axis=0))
                            nc.gpsimd.indirect_dma_start(
                                out=qg[:, jj, :], out_offset=None,
                                in_=qtab[e][:],
                                in_offset=bass.IndirectOffsetOnAxis(
                                    ap=q32[:, b2 * 7 + jj:b2 * 7 + jj + 1],
                                    axis=0))
                        prod = eg.tile([128, 7, D], f32, tag="prod")
                        nc.vector.tensor_tensor(out=prod[:], in0=kvg[:, :, 0:D],
                                                in1=qg[:], op=OP.mult)
                        score = eg.tile([128, 7, H_], f32, tag="score")
                        nc.vector.reduce_sum(
                            out=score[:],
                            in_=prod[:].rearrange("p s (h d) -> p s h d", d=DK),
                            axis=mybir.AxisListType.X)
                        w = eg.tile([128, 7, H_], f32, tag="w")
                        nc.scalar.activation(out=w[:], in_=score[:], func=AF.Exp)
                        rhsb = eg.tile([128, 7, 132], bf16, tag="rhsb")
                        nc.vector.tensor_tensor(
                            out=rhsb[:, :, 0:D].rearrange(
                                "p s (h d) -> p s h d", d=DK),
                            in0=kvg[:, :, D:2 * D].rearrange(
                                "p s (h d) -> p s h d", d=DK),
                            in1=w[:, :, :, None].to_broadcast([128, 7, H_, DK]),
                            op=OP.mult)
                        nc.vector.tensor_copy(out=rhsb[:, :, D:132], in_=w[:])
                        ohb = eg.tile([128, 7, 128], bf16, tag="ohb")
                        nc.vector.tensor_tensor(
                            out=ohb[:],
                            in0=o16[:, j0:j0 + 7, None].to_broadcast(
                                [128, 7, 128]),
                            in1=iota[:, None, :].to_broadcast([128, 7, 128]),
                            op=OP.is_equal)
                        for jj in range(7):
                            nc.tensor.matmul(
                                out=pmsg[:], lhsT=ohb[:, jj, :],
                                rhs=rhsb[:, jj, :],
                                start=(b2 == 0 and jj == 0),
                                stop=(b2 == 1 and jj == 6))
                    dens = eg.tile([128, H_], f32, tag="dens")
                    nc.vector.tensor_scalar_max(out=dens[:], in0=pmsg[:, D:132],
                                                scalar1=1e-30)
                    rec = eg.tile([128, H_], f32, tag="rec")
                    nc.vector.reciprocal(out=rec[:], in_=dens[:])
                    ho = eg.tile([128, 128], f16, tag="ho")
                    nc.vector.tensor_tensor(
                        out=ho[:].rearrange("p (h d) -> p h d", d=DK),
                        in0=pmsg[:, 0:D].rearrange("p (h d) -> p h d", d=DK),
                        in1=rec[:, :, None].to_broadcast([128, H_, DK]),
                        op=OP.mult)
                    nc.sync.dma_start(out=htab[e][d * 128:(d + 1) * 128, :],
                                      in_=ho[:])

            phase_b("ui", 0)
            phase_b("iu", 1)

            # ---------------- Phase C: node output + LN ----------------
            def phase_c(e, nt, wa, al, g, b):
                for t in range(DT):
                    hT = wk.tile([128, 128], f16, tag="hT")
                    nc.sync.dma_start(out=hT[:],
                                      in_=htab[e][t * 128:(t + 1) * 128, :],
                                      transpose=True)
                    ps = psA.tile([128, 128], f32, space="PSUM", tag="psa",
                                  bufs=3, name="psC")
                    nc.tensor.matmul(out=ps[:], lhsT=hT[:], rhs=Wsb[wa][:],
                                     start=True, stop=True)
                    fin = wk.tile([128, 128], f16, tag="finC")
                    nc.sync.dma_start(out=fin[:],
                                      in_=io[nt, t * 128:(t + 1) * 128, :])
                    fm = wk.tile([128, 128], f32, tag="fm")
                    nc.vector.tensor_scalar_mul(out=fm[:], in0=fin[:],
                                                scalar1=alt[al][:, 0:1])
                    lin = wk.tile([128, 128], f32, tag="lin")
                    nc.vector.tensor_tensor(out=lin[:], in0=ps[:], in1=fm[:],
                                            op=OP.add)
                    nc.vector.tensor_tensor(out=lin[:], in0=lin[:],
                                            in1=Bsb[f"bam_{al[1]}"][:],
                                            op=OP.add)
                    s = wk.tile([128, 1], f32, tag="s")
                    nc.vector.reduce_sum(out=s[:], in_=lin[:],
                                         axis=mybir.AxisListType.X)
                    mn = wk.tile([128, 1], f32, tag="mn")
                    nc.vector.tensor_scalar_mul(out=mn[:], in0=s[:],
                                                scalar1=-1.0 / D)
                    xc = wk.tile([128, 128], f32, tag="xc")
                    nc.vector.tensor_scalar_add(out=xc[:], in0=lin[:],
                                                scalar1=mn[:, 0:1])
                    sq = wk.tile([128, 128], f32, tag="sq")
                    nc.vector.tensor_tensor(out=sq[:], in0=xc[:], in1=xc[:],
                                            op=OP.mult)
                    vs = wk.tile([128, 1], f32, tag="vs")
                    nc.vector.reduce_sum(out=vs[:], in_=sq[:],
                                         axis=mybir.AxisListType.X)
                    sd = wk.tile([128, 1], f32, tag="sd")
                    nc.scalar.activation(out=sd[:], in_=vs[:], func=AF.Sqrt,
                                         scale=1.0 / D, bias=epst[:, 0:1])
                    rs = wk.tile([128, 1], f32, tag="rs")
                    nc.vector.reciprocal(out=rs[:], in_=sd[:])
                    y = wk.tile([128, 128], f32, tag="y")
                    nc.vector.tensor_scalar_mul(out=y[:], in0=xc[:],
                                                scalar1=rs[:, 0:1])
                    nc.vector.tensor_tensor(out=y[:], in0=y[:], in1=Bsb[g][:],
                                            op=OP.mult)
                    y16 = wk.tile([128, 128], f16, tag="y16")
                    nc.vector.tensor_tensor(out=y16[:], in0=y[:], in1=Bsb[b][:],
                                            op=OP.add)
                    nc.sync.dma_start(out=io[nt, t * 128:(t + 1) * 128, :],
                                      in_=y16[:])

            phase_c("ui", 1, "Wa_i", "ai", "g_i", "b_i")  # items get ui msgs
            phase_c("iu", 0, "Wa_u", "au", "g_u", "b_u")

    nc.compile()

    # ---- jitted SPMD runner: operands (wd, sidx, oidx, io[donated]) ----
    b2j.install_neuronx_cc_hook()

    partition_name = (nc.partition_id_tensor.name
                      if nc.partition_id_tensor else None)
    in_names, out_names, out_avals = [], [], []
    for alloc in nc.m.functions[0].allocations:
        if not isinstance(alloc, mybir.MemoryLocationSet):
            continue
        name = alloc.memorylocations[0].name
        if alloc.kind == "ExternalInput":
            if name != partition_name:
                in_names.append(name)
        elif alloc.kind == "ExternalOutput":
            shape = tuple(alloc.tensor_shape)
            out_names.append(name)
            out_avals.append(
                jax.core.ShapedArray(shape, mybir.dt.np(alloc.dtype)))
    assert in_names == ["wd", "sidx", "oidx"], in_names
    assert out_names == ["io"], out_names
    all_names = in_names + out_names
    if partition_name is not None:
        all_names.append(partition_name)

    def _body(*args):
        operands = list(args)
        if partition_name is not None:
            operands.append(b2j.partition_id_tensor())
        outs = b2j._bass_exec_p.bind(
            *operands,
            out_avals=tuple(out_avals),
            in_names=tuple(all_names),
            out_names=tuple(out_names),
            lowering_input_output_aliases=(),
            sim_require_finite=True,
            sim_require_nnan=True,
            nc=nc,
        )
        return tuple(outs)

    devices = jax.devices()[:C]
    mesh = Mesh(np.asarray(devices), ("core",))
    specs = (PartitionSpec("core"),) * 4
    fn = jax.jit(
        shard_map(_body, mesh=mesh, in_specs=specs,
                  out_specs=(PartitionSpec("core"),), check_rep=False),
        donate_argnums=(3,), keep_unused=True)
    args = (np.zeros(C * WD_LEN, np.float16),
            np.zeros((C * 128, 2 * T), np.int32),
            np.zeros((C * 128, 2 * T), np.uint8),
            np.zeros((C * 2, NLP, D), np.float16))
    return fn.lower(*args).compile()


def _get_compiled():
    """Load the compiled SPMD executable from the disk cache, else build."""
    import os
    import pickle
    from jax.experimental.serialize_executable import (
        serialize, deserialize_and_load)
    try:
        with open(_CACHE_FILE, "rb") as f:
            payload, in_tree, out_tree = pickle.load(f)
        return deserialize_and_load(payload, in_tree, out_tree)
    except Exception:
        pass
    compiled = _build()
    try:
        os.makedirs(os.path.dirname(_CACHE_FILE), exist_ok=True)
        with open(_CACHE_FILE + ".tmp", "wb") as f:
            pickle.dump(serialize(compiled), f)
        os.replace(_CACHE_FILE + ".tmp", _CACHE_FILE)
    except Exception:
        pass
    return compiled


# ------------------------------------------------------------------ kernel
def kernel(feats_user, feats_item, src_ui, dst_ui, src_iu, dst_iu,
           Wk_u, bk_u, Wq_u, bq_u, Wv_u, bv_u, Wa_u, ba_u, lng_u, lnb_u, skip_u,
           Wk_i, bk_i, Wq_i, bq_i, Wv_i, bv_i, Wa_i, ba_i, lng_i, lnb_i, skip_i,
           mu_ui, watt_ui, wmsg_ui, mu_iu, watt_iu, wmsg_iu):
    global _BUILT
    import jax
    from jax.sharding import Mesh, PartitionSpec, NamedSharding

    f32, f16 = np.float32, np.float16
    fU = np.asarray(feats_user, f32)
    fI = np.asarray(feats_item, f32)

    if _BUILT is None:
        _BUILT = _get_compiled()

    # start the big feats upload first; host prep overlaps with the transfer
    io_g = np.zeros((C, 2, NLP, D), f16)
    io_g[:, 0, :NL] = fU.reshape(C, NL, D).astype(f16)
    io_g[:, 1, :NL] = fI.reshape(C, NL, D).astype(f16)
    io_g = io_g.reshape(C * 2, NLP, D)
    mesh = Mesh(np.asarray(jax.devices()[:C]), ("core",))
    sh = NamedSharding(mesh, PartitionSpec("core"))
    io_dev = jax.device_put(io_g, sh)  # async

    wdv = np.zeros(WD_LEN, f32)
    mats, rows = {}, {}
    for e, (Wk, bk, Wv, bv, Wq, bq, watt, wmsg, mu) in {
        "ui": (Wk_u, bk_u, Wv_u, bv_u, Wq_i, bq_i, watt_ui, wmsg_ui, mu_ui),
        "iu": (Wk_i, bk_i, Wv_i, bv_i, Wq_u, bq_u, watt_iu, wmsg_iu, mu_iu),
    }.items():
        Ba = _blockdiag(np.asarray(watt, f32))
        Bm = _blockdiag(np.asarray(wmsg, f32))
        qs = np.repeat(np.asarray(mu, f32) / math.sqrt(DK), DK)
        mats[f"Wk_{e}"] = np.asarray(Wk, f32) @ Ba
        mats[f"Wv_{e}"] = np.asarray(Wv, f32) @ Bm
        mats[f"Wq_{e}"] = np.asarray(Wq, f32) * qs[None, :]
        rows[f"bk_{e}"] = np.asarray(bk, f32) @ Ba
        rows[f"bv_{e}"] = np.asarray(bv, f32) @ Bm
        rows[f"bq_{e}"] = np.asarray(bq, f32) * qs

    al_u = 1.0 / (1.0 + math.exp(-float(np.asarray(skip_u).reshape(-1)[0])))
    al_i = 1.0 / (1.0 + math.exp(-float(np.asarray(skip_i).reshape(-1)[0])))
    mats["Wa_u"] = al_u * np.asarray(Wa_u, f32)
    mats["Wa_i"] = al_i * np.asarray(Wa_i, f32)
    rows["g_u"] = np.asarray(lng_u, f32); rows["b_u"] = np.asarray(lnb_u, f32)
    rows["g_i"] = np.asarray(lng_i, f32); rows["b_i"] = np.asarray(lnb_i, f32)
    rows["bam_u"] = al_u * np.asarray(ba_u, f32)
    rows["bam_i"] = al_i * np.asarray(ba_i, f32)

    WNAMES = ("Wk_ui", "Wv_ui", "Wq_ui", "Wk_iu", "Wv_iu", "Wq_iu",
              "Wa_u", "Wa_i")
    RNAMES = ("bk_ui", "bv_ui", "bq_ui", "bk_iu", "bv_iu", "bq_iu",
              "g_u", "b_u", "g_i", "b_i", "bam_u", "bam_i")
    for m, n in enumerate(WNAMES):
        wdv[m * WMAT:(m + 1) * WMAT] = mats[n].ravel()
    for r, n in enumerate(RNAMES):
        wdv[R0 + r * D:R0 + (r + 1) * D] = rows[n]
    wdv[R0 + 12 * D] = 1.0 - al_u
    wdv[R0 + 12 * D + 1] = 1.0 - al_i
    wd16 = wdv.astype(f16)

    s_ui, o_ui = _edge_prep(np.asarray(src_ui), np.asarray(dst_ui))
    s_iu, o_iu = _edge_prep(np.asarray(src_iu), np.asarray(dst_iu))

    # global (concat-over-cores) arrays for the sharded jit
    wd_g = np.ascontiguousarray(
        np.broadcast_to(wd16, (C, WD_LEN))).reshape(C * WD_LEN)
    sidx_g = np.ascontiguousarray(
        np.concatenate([s_ui, s_iu], axis=2)).reshape(C * 128, 2 * T)
    oidx_g = np.ascontiguousarray(
        np.concatenate([o_ui, o_iu], axis=2)).reshape(C * 128, 2 * T)

    (res,) = _BUILT(wd_g, sidx_g, oidx_g, io_dev)
    res = np.asarray(res).reshape(C, 2, NLP, D)

    out = np.empty((2, N, D), f32)
    out[0] = res[:, 0, :NL].reshape(N, D)
    out[1] = res[:, 1, :NL].reshape(N, D)
    return out
